# revision 12
# baseline (speedup 1.0000x reference)
"""Wilson-Dirac operator on Trainium2, 8 NeuronCores, T-axis domain decomposition.

v2: SoA fp16 "plane" layout. Each (spin,color,ri) component of psi (24 planes),
each (ri,row,col) component of -0.5*U (18 planes/mu), and each output component
(24 planes) is a separate contiguous plane of [t, z] (z innermost) per (x,y)
lattice row. All VectorE tensor_tensor ops then stream with innermost stride 1
over 144-elem (or larger) contiguous spans in fp16, which hits the DVE 2x_1p
packed mode (2 elem/lane/cycle) - fp32 tensor_tensor is capped at 1x.

Per direction mu, sign:
  proj      h[j,b,ri] = psi[A] + c_j psi[B]           (1-2 ops per j)
  products  P[j,th,tu,b,a] = U'[tu,(b,a)] * h[j,th,b] (8 ops, U read plain for
            bwd / AP-transposed for fwd - one shipped gauge layout serves both)
  bsum      P[...,b0,:] += P[...,b1,:] += P[...,b2,:] (2 ops)
  combine   m = (P00 +- P11) + i(P10 -+ P01)          (2 ops)
  expand    out[s] += {m[j], +-m[e], +-i m[e]}        (3-5 ops)

Shift handling:
  t: free-dim offsets into the inline t-halo (psi slots t0-1..t0+TS, gauge
     slots t0-1..t0+TS-1); x: DRAM loads of row-shifted psi/gauge rows (full
     contiguous rows); y: compute at source + SBUF->SBUF DMA row shift of m
     (fwd) / h (bwd) - 12 planes instead of 24+18; z: compute at source +
     in-SBUF shifted copy of m (fwd) / h (bwd) with periodic wrap split.

Host pre-scales gauge by -0.5 and ships ONE SoA layout (fwd reads it
transposed via AP strides). All engine APs keep <= 3 free dims. The
one-sync-wait-per-instruction walrus limit is handled by _split_waits_json."""

import numpy as np

# ---------------------------------------------------------------- constants
X = Y = 24
Z = 24
T = 48
NCORES = 8
TS = T // NCORES          # 6 t-slices per core
TH = TS + 2               # psi t slots (with halo both sides)
TG = TS + 1               # gauge t slots (halo at t0-1)
S = TS * Z                # 144: work-plane span [t,z]
PP = TH * Z               # 192: psi plane span
GP = TG * Z               # 168: gauge plane span
XY = X * Y
MASSP4 = 4.5

# h_j = psi[j] + c_j * psi[B_j]; expansion: out[0]+=m[0], out[1]+=m[1],
# out[2] += d0*m[e0], out[3] += d1*m[e1].  Backward: c -> -c, d -> -d.
DIRSPEC = {
    0: dict(B=(3, 2), c=(-1j, -1j), e=(1, 0), d=(+1j, +1j)),
    1: dict(B=(3, 2), c=(-1, +1),   e=(1, 0), d=(+1, -1)),
    2: dict(B=(2, 3), c=(-1j, +1j), e=(0, 1), d=(+1j, -1j)),
    3: dict(B=(2, 3), c=(+1, +1),   e=(0, 1), d=(+1, +1)),
}

_CACHE = {}


def _split_waits_json(raw: bytes) -> bytes:
    """Walrus here allows only ONE sync-wait per instruction. Keep the last
    wait on the instruction, hoist the rest onto NoOps inserted immediately
    before it (same engine, semaphores monotonic => exact)."""
    import json
    bj = json.loads(raw)
    nid = 0
    for fn in bj.get("functions", []):
        for bb in fn.get("blocks", []):
            out = []
            changed = False
            for inst in bb.get("instructions", []):
                si = inst.get("sync_info")
                ow = (si or {}).get("on_wait") or []
                if len(ow) > 1:
                    changed = True
                    for w in ow[:-1]:
                        nid += 1
                        out.append({
                            "engine": inst["engine"], "ins": [], "outs": [],
                            "name": f"WSPL-{nid}", "opcode": "NoOp",
                            "sync_info": {"on_update": [], "on_wait": [w]},
                        })
                    si["on_wait"] = [ow[-1]]
                out.append(inst)
            if changed:
                bb["instructions"] = out
    return json.dumps(bj).encode()


def _install_json_wait_fix():
    import concourse.bass as bass
    if getattr(bass.Bass, "_wd_wait_fix", False):
        return
    orig = bass.Bass.to_json_bytes

    def patched(self, *a, **k):
        return _split_waits_json(orig(self, *a, **k))

    bass.Bass.to_json_bytes = patched
    bass.Bass._wd_wait_fix = True


def build_module(NXC=5):
    import concourse.bass as bass
    import concourse.mybir as mybir
    from concourse.ap import AP
    from concourse.mybir import AluOpType
    from concourse.tile import TileContext

    _install_json_wait_fix()

    F16 = mybir.dt.float16

    nc = bass.Bass()
    fh = nc.declare_dram_parameter("fh", [XY, 24 * PP], F16, isOutput=False)
    gg = nc.declare_dram_parameter("gg", [4, XY, 18 * GP], F16, isOutput=False)
    outp = nc.declare_dram_parameter("outp", [XY, 24 * S], F16, isOutput=True)

    def sap(t, off, dims):
        return AP(t.tensor, t.offset + off, [list(t.ap[0])] + [list(d) for d in dims])

    with TileContext(nc) as tc:
        ctx_pool = tc.tile_pool(name="work", bufs=1)
        pool = ctx_pool.__enter__()
        V = nc.vector
        A = AluOpType

        def dma(out, in_):
            nc.sync.dma_start(out=out, in_=in_)

        for x0 in range(0, X, NXC):
            nx = min(NXC, X - x0)
            R = nx * Y
            r0 = x0 * Y

            # ---------------- DMA loads (big contiguous row blocks) --------
            psi_al = pool.tile([R, 24 * PP], F16, tag="psi", bufs=2)
            dma(psi_al[:], fh[r0:r0 + R])

            def load_rows(tag, src, drow, nelem, bufs):
                tl = pool.tile([R, nelem], F16, tag=tag, bufs=bufs)
                rs = (r0 + drow) % XY
                if rs + R <= XY:
                    dma(tl[:], src(rs, rs + R))
                else:
                    n1 = XY - rs
                    dma(tl[0:n1], src(rs, XY))
                    dma(tl[n1:R], src(0, R - n1))
                return tl

            g_loc = []
            for mu in range(4):
                g_loc.append(load_rows(f"g{mu}", lambda a, b, mu=mu: gg[mu, a:b],
                                       0, 18 * GP, 2))
            g_xf = load_rows("gxf", lambda a, b: gg[0, a:b], -Y, 18 * GP, 2)
            psi_xf = load_rows("pxf", lambda a, b: fh[a:b], -Y, 24 * PP, 2)
            psi_xb = load_rows("pxb", lambda a, b: fh[a:b], +Y, 24 * PP, 2)

            out_t = pool.tile([R, 24 * S], F16, tag="out", bufs=2)

            # ---------------- mass term (DVE tensor_scalar, 4x mode) -------
            V.tensor_scalar_mul(
                sap(out_t, 0, [[S, 24], [1, S]]),
                sap(psi_al, Z, [[PP, 24], [1, S]]),
                MASSP4)

            # ---------------- op emitters ----------------------------------
            def proj(psi_t, toff, B, cj):
                """h[j,b,ri] = psi[A=j] + c_j psi[B_j]; returns h tile."""
                ht = pool.tile([R, 12 * S], F16, tag="h", bufs=4)
                jB = (B[1] - B[0]) * 6 * PP
                if cj[0] == cj[1] and cj[0].imag == 0.0:
                    op = A.add if cj[0].real > 0 else A.subtract
                    V.tensor_tensor(
                        sap(ht, 0, [[6 * S, 2], [S, 6], [1, S]]),
                        sap(psi_t, toff, [[6 * PP, 2], [PP, 6], [1, S]]),
                        sap(psi_t, B[0] * 6 * PP + toff, [[jB, 2], [PP, 6], [1, S]]),
                        op)
                elif cj[0] == cj[1]:
                    sg = cj[0].imag > 0
                    V.tensor_tensor(
                        sap(ht, 0, [[6 * S, 2], [2 * S, 3], [1, S]]),
                        sap(psi_t, toff, [[6 * PP, 2], [2 * PP, 3], [1, S]]),
                        sap(psi_t, B[0] * 6 * PP + PP + toff,
                            [[jB, 2], [2 * PP, 3], [1, S]]),
                        A.subtract if sg else A.add)
                    V.tensor_tensor(
                        sap(ht, S, [[6 * S, 2], [2 * S, 3], [1, S]]),
                        sap(psi_t, PP + toff, [[6 * PP, 2], [2 * PP, 3], [1, S]]),
                        sap(psi_t, B[0] * 6 * PP + toff,
                            [[jB, 2], [2 * PP, 3], [1, S]]),
                        A.add if sg else A.subtract)
                    return ht
                else:
                    for j in (0, 1):
                        c = cj[j]
                        ab = j * 6 * PP + toff
                        bb_ = B[j] * 6 * PP + toff
                        if c.imag == 0.0:
                            op = A.add if c.real > 0 else A.subtract
                            V.tensor_tensor(
                                sap(ht, j * 6 * S, [[S, 6], [1, S]]),
                                sap(psi_t, ab, [[PP, 6], [1, S]]),
                                sap(psi_t, bb_, [[PP, 6], [1, S]]), op)
                        else:
                            sg = c.imag > 0
                            # h_re = psiA_re -+ psiB_im ; h_im = psiA_im +- psiB_re
                            V.tensor_tensor(
                                sap(ht, j * 6 * S, [[2 * S, 3], [1, S]]),
                                sap(psi_t, ab, [[2 * PP, 3], [1, S]]),
                                sap(psi_t, bb_ + PP, [[2 * PP, 3], [1, S]]),
                                A.subtract if sg else A.add)
                            V.tensor_tensor(
                                sap(ht, j * 6 * S + S, [[2 * S, 3], [1, S]]),
                                sap(psi_t, ab + PP, [[2 * PP, 3], [1, S]]),
                                sap(psi_t, bb_, [[2 * PP, 3], [1, S]]),
                                A.add if sg else A.subtract)
                return ht

            def su3_front(g_t, gtoff, transposed, ht):
                """products + bsum into P tile."""
                pt = pool.tile([R, 72 * S], F16, tag="P", bufs=2)
                if transposed:
                    gdims = [[3 * GP, 3], [GP, 3], [1, S]]     # read U[b,a]
                else:
                    gdims = [[GP, 3], [3 * GP, 3], [1, S]]     # read U[a,b]
                for j in (0, 1):
                    for th in (0, 1):
                        for tu in (0, 1):
                            V.tensor_tensor(
                                sap(pt, (j * 36 + th * 18 + tu * 9) * S,
                                    [[3 * S, 3], [S, 3], [1, S]]),
                                sap(g_t, tu * 9 * GP + gtoff, gdims),
                                sap(ht, (j * 6 + th) * S, [[2 * S, 3], [0, 3], [1, S]]),
                                A.mult)
                bdims = [[9 * S, 8], [S, 3], [1, S]]
                V.tensor_tensor(sap(pt, 0, bdims), sap(pt, 0, bdims),
                                sap(pt, 3 * S, bdims), A.add)
                V.tensor_tensor(sap(pt, 0, bdims), sap(pt, 0, bdims),
                                sap(pt, 6 * S, bdims), A.add)
                return pt

            def su3_comb(pt, fwd):
                """combine P -> m tile (12 planes)."""
                mt = pool.tile([R, 12 * S], F16, tag="m", bufs=3)
                mdims = [[6 * S, 2], [2 * S, 3], [1, S]]
                pdims = [[36 * S, 2], [S, 3], [1, S]]
                V.tensor_tensor(sap(mt, 0, mdims), sap(pt, 0, pdims),
                                sap(pt, 27 * S, pdims), A.add if fwd else A.subtract)
                V.tensor_tensor(sap(mt, S, mdims), sap(pt, 18 * S, pdims),
                                sap(pt, 9 * S, pdims), A.subtract if fwd else A.add)
                return mt

            def expand(mt, e, dj):
                """out[s0,s1] += m; out[2+si] += d_si * m[e_si]."""
                V.tensor_tensor(sap(out_t, 0, [[1, 12 * S]]),
                                sap(out_t, 0, [[1, 12 * S]]),
                                sap(mt, 0, [[1, 12 * S]]), A.add)
                for si in (0, 1):
                    ob = (12 + si * 6) * S
                    eb = e[si] * 6 * S
                    dv = dj[si]
                    if dv.imag == 0.0:
                        op = A.add if dv.real > 0 else A.subtract
                        V.tensor_tensor(sap(out_t, ob, [[1, 6 * S]]),
                                        sap(out_t, ob, [[1, 6 * S]]),
                                        sap(mt, eb, [[1, 6 * S]]), op)
                    else:
                        sg = dv.imag > 0
                        # out_re += -sg*m_im ; out_im += sg*m_re
                        ore = [[2 * S, 3], [1, S]]
                        V.tensor_tensor(sap(out_t, ob, ore), sap(out_t, ob, ore),
                                        sap(mt, eb + S, ore),
                                        A.subtract if sg else A.add)
                        V.tensor_tensor(sap(out_t, ob + S, ore),
                                        sap(out_t, ob + S, ore),
                                        sap(mt, eb, ore),
                                        A.add if sg else A.subtract)

            def zshift(src_t, nplanes, dz, tag):
                """dst[t,z] = src[t, z+dz] (periodic), dz in {-1,+1}."""
                dt_ = pool.tile([R, nplanes * S], F16, tag=tag, bufs=2)
                C = V.tensor_copy
                if dz == +1:
                    C(sap(dt_, 0, [[S, nplanes], [Z, TS], [1, Z - 1]]),
                      sap(src_t, 1, [[S, nplanes], [Z, TS], [1, Z - 1]]))
                    C(sap(dt_, Z - 1, [[S, nplanes], [Z, TS], [1, 1]]),
                      sap(src_t, 0, [[S, nplanes], [Z, TS], [1, 1]]))
                else:
                    C(sap(dt_, 1, [[S, nplanes], [Z, TS], [1, Z - 1]]),
                      sap(src_t, 0, [[S, nplanes], [Z, TS], [1, Z - 1]]))
                    C(sap(dt_, 0, [[S, nplanes], [Z, TS], [1, 1]]),
                      sap(src_t, Z - 1, [[S, nplanes], [Z, TS], [1, 1]]))
                return dt_

            def yshift(src_t, nplanes, dy, tag):
                """dst[row (x,y)] = src[row (x, y+dy)] (periodic in y), via
                SBUF->SBUF DMA row shifts."""
                dt_ = pool.tile([R, nplanes * S], F16, tag=tag, bufs=2)
                D = nc.scalar.dma_start
                for g in range(nx):
                    b = g * Y
                    if dy == +1:
                        D(out=dt_[b:b + Y - 1], in_=src_t[b + 1:b + Y])
                        D(out=dt_[b + Y - 1:b + Y], in_=src_t[b:b + 1])
                    else:
                        D(out=dt_[b + 1:b + Y], in_=src_t[b:b + Y - 1])
                        D(out=dt_[b:b + 1], in_=src_t[b + Y - 1:b + Y])
                return dt_

            # ---------------- directions: software-pipelined stages -------
            # stage i emits: proj(i+2) | products+bsum(i) | comb+shift(i-1)
            # | expand(i-2), so DVE never stalls on shift DMAs or RMW chains.
            dirs_order = [(2, +1), (2, -1), (3, +1), (3, -1),
                          (1, +1), (1, -1), (0, +1), (0, -1)]
            st = {}

            def make_h(i):
                mu, sgn = dirs_order[i]
                spec = DIRSPEC[mu]
                fwd = sgn == +1
                cj = spec["c"] if fwd else tuple(-v for v in spec["c"])
                if mu == 2:
                    ht = proj(psi_al, Z, spec["B"], cj)
                    if not fwd:
                        ht = zshift(ht, 12, +1, "hsh")
                elif mu == 3:
                    ht = proj(psi_al, 0 if fwd else 2 * Z, spec["B"], cj)
                elif mu == 1:
                    ht = proj(psi_al, Z, spec["B"], cj)
                    if not fwd:
                        ht = yshift(ht, 12, +1, "hsh")
                else:
                    ht = proj(psi_xf if fwd else psi_xb, Z, spec["B"], cj)
                st[("h", i)] = ht

            def front(i):
                mu, sgn = dirs_order[i]
                fwd = sgn == +1
                if mu == 2:
                    g_t, gtoff = g_loc[2], Z
                elif mu == 3:
                    g_t, gtoff = g_loc[3], (0 if fwd else Z)
                elif mu == 1:
                    g_t, gtoff = g_loc[1], Z
                else:
                    g_t, gtoff = (g_xf if fwd else g_loc[0]), Z
                st[("p", i)] = su3_front(g_t, gtoff, fwd, st[("h", i)])

            def comb_shift(i):
                mu, sgn = dirs_order[i]
                fwd = sgn == +1
                mt = su3_comb(st[("p", i)], fwd)
                if fwd and mu == 2:
                    mt = zshift(mt, 12, -1, "msh")
                elif fwd and mu == 1:
                    mt = yshift(mt, 12, -1, "msh")
                st[("m", i)] = mt

            def expand_d(i):
                mu, sgn = dirs_order[i]
                spec = DIRSPEC[mu]
                dj = spec["d"] if sgn == +1 else tuple(-v for v in spec["d"])
                expand(st[("m", i)], spec["e"], dj)

            make_h(0)
            make_h(1)
            for i in range(8):
                if i + 2 < 8:
                    make_h(i + 2)
                front(i)
                if i >= 1:
                    comb_shift(i - 1)
                if i >= 2:
                    expand_d(i - 2)
            comb_shift(7)
            expand_d(6)
            expand_d(7)

            nc.scalar.dma_start(out=outp[r0:r0 + R], in_=out_t[:])
        ctx_pool.__exit__(None, None, None)
    return nc


# ---------------------------------------------------------------- host side
def _prep_core_inputs(fv, gv, t0):
    """fv: [X,Y,Z,T,3,4,2] f32 (c,s,ri). gv: [4,X,Y,Z,T,3,3,2] (r,c,ri).
    Returns fh [XY, 24*(TH*Z)] planes (s,c,ri) layout [t,z], and
    gg [4, XY, 18*(TG*Z)] planes (ri,r,c) of -0.5*U, both fp16."""
    Tl = T
    slots = [(t0 - 1) % Tl] + [(t0 + i) % Tl for i in range(TS)] + [(t0 + TS) % Tl]
    f = fv[:, :, :, slots]                       # [X,Y,Z,TH,c,s,ri]
    f = f.transpose(0, 1, 5, 4, 6, 3, 2)         # [X,Y,s,c,ri,TH,Z]
    fhn = np.ascontiguousarray(f, dtype=np.float16).reshape(XY, 24 * PP)
    gslots = [(t0 - 1 + i) % Tl for i in range(TG)]
    g = gv[:, :, :, :, gslots]                   # [4,X,Y,Z,TG,r,c,ri]
    g = g.transpose(0, 1, 2, 7, 5, 6, 4, 3)      # [4,X,Y,ri,r,c,TG,Z]
    ggn = np.ascontiguousarray(g, dtype=np.float32)
    ggn *= -0.5
    return fhn, ggn.astype(np.float16).reshape(4, XY, 18 * GP)


def _out_to_complex(o):
    o = o.astype(np.float32).reshape(X, Y, 4, 3, 2, TS, Z)   # [X,Y,s,c,ri,t,z]
    o = o.transpose(0, 1, 6, 5, 3, 2, 4)                     # [X,Y,Z,t,c,s,ri]
    return (o[..., 0] + 1j * o[..., 1]).astype(np.complex64)


def _build_in_maps(field, gauge_field):
    fv = np.ascontiguousarray(field).view(np.float32).reshape(X, Y, Z, T, 3, 4, 2)
    gv = np.ascontiguousarray(gauge_field).view(np.float32).reshape(4, X, Y, Z, T, 3, 3, 2)
    in_maps = []
    for k in range(NCORES):
        fhn, ggn = _prep_core_inputs(fv, gv, k * TS)
        in_maps.append({"fh": fhn, "gg": ggn})
    return in_maps


def kernel(field, gauge_field):
    from concourse.bass_utils import run_bass_kernel_spmd

    key = "full"
    if key not in _CACHE:
        _CACHE[key] = build_module()
    nc = _CACHE[key]

    in_maps = _build_in_maps(field, gauge_field)
    res = run_bass_kernel_spmd(nc, in_maps, list(range(NCORES))).results

    out = np.empty((X, Y, Z, T, 3, 4), np.complex64)
    for k in range(NCORES):
        out[:, :, :, k * TS:(k + 1) * TS] = _out_to_complex(res[k]["outp"])
    return out


# revision 14
# speedup vs baseline: 1.0263x; 1.0263x over previous
"""Wilson-Dirac operator on Trainium2, 8 NeuronCores, T-axis domain decomposition.

v2: SoA fp16 "plane" layout. Each (spin,color,ri) component of psi (24 planes),
each (ri,row,col) component of -0.5*U (18 planes/mu), and each output component
(24 planes) is a separate contiguous plane of [t, z] (z innermost) per (x,y)
lattice row. All VectorE tensor_tensor ops then stream with innermost stride 1
over 144-elem (or larger) contiguous spans in fp16, which hits the DVE 2x_1p
packed mode (2 elem/lane/cycle) - fp32 tensor_tensor is capped at 1x.

Per direction mu, sign:
  proj      h[j,b,ri] = psi[A] + c_j psi[B]           (1-2 ops per j)
  products  P[j,th,tu,b,a] = U'[tu,(b,a)] * h[j,th,b] (8 ops, U read plain for
            bwd / AP-transposed for fwd - one shipped gauge layout serves both)
  bsum      P[...,b0,:] += P[...,b1,:] += P[...,b2,:] (2 ops)
  combine   m = (P00 +- P11) + i(P10 -+ P01)          (2 ops)
  expand    out[s] += {m[j], +-m[e], +-i m[e]}        (3-5 ops)

Shift handling:
  t: free-dim offsets into the inline t-halo (psi slots t0-1..t0+TS, gauge
     slots t0-1..t0+TS-1); x: DRAM loads of row-shifted psi/gauge rows (full
     contiguous rows); y: compute at source + SBUF->SBUF DMA row shift of m
     (fwd) / h (bwd) - 12 planes instead of 24+18; z: compute at source +
     in-SBUF shifted copy of m (fwd) / h (bwd) with periodic wrap split.

Host pre-scales gauge by -0.5 and ships ONE SoA layout (fwd reads it
transposed via AP strides). All engine APs keep <= 3 free dims. The
one-sync-wait-per-instruction walrus limit is handled by _split_waits_json."""

import numpy as np

# ---------------------------------------------------------------- constants
X = Y = 24
Z = 24
T = 48
NCORES = 8
TS = T // NCORES          # 6 t-slices per core
TH = TS + 2               # psi t slots (with halo both sides)
TG = TS + 1               # gauge t slots (halo at t0-1)
S = TS * Z                # 144: work-plane span [t,z]
PP = TH * Z               # 192: psi plane span
GP = TG * Z               # 168: gauge plane span
XY = X * Y
MASSP4 = 4.5

# h_j = psi[j] + c_j * psi[B_j]; expansion: out[0]+=m[0], out[1]+=m[1],
# out[2] += d0*m[e0], out[3] += d1*m[e1].  Backward: c -> -c, d -> -d.
DIRSPEC = {
    0: dict(B=(3, 2), c=(-1j, -1j), e=(1, 0), d=(+1j, +1j)),
    1: dict(B=(3, 2), c=(-1, +1),   e=(1, 0), d=(+1, -1)),
    2: dict(B=(2, 3), c=(-1j, +1j), e=(0, 1), d=(+1j, -1j)),
    3: dict(B=(2, 3), c=(+1, +1),   e=(0, 1), d=(+1, +1)),
}

_CACHE = {}


def _split_waits_json(raw: bytes) -> bytes:
    """Walrus here allows only ONE sync-wait per instruction. Keep the last
    wait on the instruction, hoist the rest onto NoOps inserted immediately
    before it (same engine, semaphores monotonic => exact)."""
    import json
    bj = json.loads(raw)
    nid = 0
    for fn in bj.get("functions", []):
        for bb in fn.get("blocks", []):
            out = []
            changed = False
            for inst in bb.get("instructions", []):
                si = inst.get("sync_info")
                ow = (si or {}).get("on_wait") or []
                if len(ow) > 1:
                    changed = True
                    for w in ow[:-1]:
                        nid += 1
                        out.append({
                            "engine": inst["engine"], "ins": [], "outs": [],
                            "name": f"WSPL-{nid}", "opcode": "NoOp",
                            "sync_info": {"on_update": [], "on_wait": [w]},
                        })
                    si["on_wait"] = [ow[-1]]
                out.append(inst)
            if changed:
                bb["instructions"] = out
    return json.dumps(bj).encode()


def _install_json_wait_fix():
    import concourse.bass as bass
    if getattr(bass.Bass, "_wd_wait_fix", False):
        return
    orig = bass.Bass.to_json_bytes

    def patched(self, *a, **k):
        return _split_waits_json(orig(self, *a, **k))

    bass.Bass.to_json_bytes = patched
    bass.Bass._wd_wait_fix = True


def build_module(NXC=5):
    import concourse.bass as bass
    import concourse.mybir as mybir
    from concourse.ap import AP
    from concourse.mybir import AluOpType
    from concourse.tile import TileContext

    _install_json_wait_fix()

    F16 = mybir.dt.float16

    nc = bass.Bass()
    fh = nc.declare_dram_parameter("fh", [XY, 24 * PP], F16, isOutput=False)
    gg = nc.declare_dram_parameter("gg", [4, XY, 18 * GP], F16, isOutput=False)
    outp = nc.declare_dram_parameter("outp", [XY, 24 * S], F16, isOutput=True)

    def sap(t, off, dims):
        return AP(t.tensor, t.offset + off, [list(t.ap[0])] + [list(d) for d in dims])

    with TileContext(nc) as tc:
        ctx_pool = tc.tile_pool(name="work", bufs=1)
        pool = ctx_pool.__enter__()
        V = nc.vector
        A = AluOpType

        def dma(out, in_):
            nc.sync.dma_start(out=out, in_=in_)

        for x0 in range(0, X, NXC):
            nx = min(NXC, X - x0)
            R = nx * Y
            r0 = x0 * Y

            # ---------------- DMA loads (big contiguous row blocks) --------
            psi_al = pool.tile([R, 24 * PP], F16, tag="psi", bufs=2)
            dma(psi_al[:], fh[r0:r0 + R])

            def load_rows(tag, src, drow, nelem, bufs, eng=None):
                tl = pool.tile([R, nelem], F16, tag=tag, bufs=bufs)
                rs = (r0 + drow) % XY
                D_ = eng or dma
                def issue(o, i):
                    if eng is None:
                        dma(o, i)
                    else:
                        eng(out=o, in_=i)
                if rs + R <= XY:
                    issue(tl[:], src(rs, rs + R))
                else:
                    n1 = XY - rs
                    issue(tl[0:n1], src(rs, XY))
                    issue(tl[n1:R], src(0, R - n1))
                return tl

            # loads issued in consumption order: sync queue feeds the
            # mu=2,3,1 pipeline head; scalar queue preloads the late x tiles.
            g_loc = [None] * 4
            for mu in (2, 3, 1):
                g_loc[mu] = load_rows(f"g{mu}", lambda a, b, mu=mu: gg[mu, a:b],
                                      0, 18 * GP, 2)
            D = nc.scalar.dma_start
            psi_xf = load_rows("pxf", lambda a, b: fh[a:b], -Y, 24 * PP, 2, D)
            psi_xb = load_rows("pxb", lambda a, b: fh[a:b], +Y, 24 * PP, 2, D)
            g_xf = load_rows("gxf", lambda a, b: gg[0, a:b], -Y, 18 * GP, 2, D)
            g_loc[0] = load_rows("g0", lambda a, b: gg[0, a:b], 0, 18 * GP, 2, D)

            out_t = pool.tile([R, 24 * S], F16, tag="out", bufs=2)

            # ---------------- mass term on ScalarE -------------------------
            nc.scalar.mul(
                sap(out_t, 0, [[S, 24], [1, S]]),
                sap(psi_al, Z, [[PP, 24], [1, S]]),
                MASSP4)

            # ---------------- op emitters ----------------------------------
            def proj(psi_t, toff, B, cj):
                """h[j,b,ri] = psi[A=j] + c_j psi[B_j]; returns h tile."""
                ht = pool.tile([R, 12 * S], F16, tag="h", bufs=4)
                jB = (B[1] - B[0]) * 6 * PP
                if cj[0] == cj[1] and cj[0].imag == 0.0:
                    op = A.add if cj[0].real > 0 else A.subtract
                    V.tensor_tensor(
                        sap(ht, 0, [[6 * S, 2], [S, 6], [1, S]]),
                        sap(psi_t, toff, [[6 * PP, 2], [PP, 6], [1, S]]),
                        sap(psi_t, B[0] * 6 * PP + toff, [[jB, 2], [PP, 6], [1, S]]),
                        op)
                elif cj[0] == cj[1]:
                    sg = cj[0].imag > 0
                    V.tensor_tensor(
                        sap(ht, 0, [[6 * S, 2], [2 * S, 3], [1, S]]),
                        sap(psi_t, toff, [[6 * PP, 2], [2 * PP, 3], [1, S]]),
                        sap(psi_t, B[0] * 6 * PP + PP + toff,
                            [[jB, 2], [2 * PP, 3], [1, S]]),
                        A.subtract if sg else A.add)
                    V.tensor_tensor(
                        sap(ht, S, [[6 * S, 2], [2 * S, 3], [1, S]]),
                        sap(psi_t, PP + toff, [[6 * PP, 2], [2 * PP, 3], [1, S]]),
                        sap(psi_t, B[0] * 6 * PP + toff,
                            [[jB, 2], [2 * PP, 3], [1, S]]),
                        A.add if sg else A.subtract)
                    return ht
                else:
                    for j in (0, 1):
                        c = cj[j]
                        ab = j * 6 * PP + toff
                        bb_ = B[j] * 6 * PP + toff
                        if c.imag == 0.0:
                            op = A.add if c.real > 0 else A.subtract
                            V.tensor_tensor(
                                sap(ht, j * 6 * S, [[S, 6], [1, S]]),
                                sap(psi_t, ab, [[PP, 6], [1, S]]),
                                sap(psi_t, bb_, [[PP, 6], [1, S]]), op)
                        else:
                            sg = c.imag > 0
                            # h_re = psiA_re -+ psiB_im ; h_im = psiA_im +- psiB_re
                            V.tensor_tensor(
                                sap(ht, j * 6 * S, [[2 * S, 3], [1, S]]),
                                sap(psi_t, ab, [[2 * PP, 3], [1, S]]),
                                sap(psi_t, bb_ + PP, [[2 * PP, 3], [1, S]]),
                                A.subtract if sg else A.add)
                            V.tensor_tensor(
                                sap(ht, j * 6 * S + S, [[2 * S, 3], [1, S]]),
                                sap(psi_t, ab + PP, [[2 * PP, 3], [1, S]]),
                                sap(psi_t, bb_, [[2 * PP, 3], [1, S]]),
                                A.add if sg else A.subtract)
                return ht

            def su3_front(g_t, gtoff, transposed, ht):
                """products + bsum into P tile."""
                pt = pool.tile([R, 72 * S], F16, tag="P", bufs=2)
                if transposed:
                    gdims = [[3 * GP, 3], [GP, 3], [1, S]]     # read U[b,a]
                else:
                    gdims = [[GP, 3], [3 * GP, 3], [1, S]]     # read U[a,b]
                for j in (0, 1):
                    for th in (0, 1):
                        for tu in (0, 1):
                            V.tensor_tensor(
                                sap(pt, (j * 36 + th * 18 + tu * 9) * S,
                                    [[3 * S, 3], [S, 3], [1, S]]),
                                sap(g_t, tu * 9 * GP + gtoff, gdims),
                                sap(ht, (j * 6 + th) * S, [[2 * S, 3], [0, 3], [1, S]]),
                                A.mult)
                bdims = [[9 * S, 8], [S, 3], [1, S]]
                V.tensor_tensor(sap(pt, 0, bdims), sap(pt, 0, bdims),
                                sap(pt, 3 * S, bdims), A.add)
                V.tensor_tensor(sap(pt, 0, bdims), sap(pt, 0, bdims),
                                sap(pt, 6 * S, bdims), A.add)
                return pt

            def su3_comb(pt, fwd):
                """combine P -> m tile (12 planes)."""
                mt = pool.tile([R, 12 * S], F16, tag="m", bufs=3)
                mdims = [[6 * S, 2], [2 * S, 3], [1, S]]
                pdims = [[36 * S, 2], [S, 3], [1, S]]
                V.tensor_tensor(sap(mt, 0, mdims), sap(pt, 0, pdims),
                                sap(pt, 27 * S, pdims), A.add if fwd else A.subtract)
                V.tensor_tensor(sap(mt, S, mdims), sap(pt, 18 * S, pdims),
                                sap(pt, 9 * S, pdims), A.subtract if fwd else A.add)
                return mt

            def expand(mt, e, dj):
                """out[s0,s1] += m; out[2+si] += d_si * m[e_si]."""
                V.tensor_tensor(sap(out_t, 0, [[1, 12 * S]]),
                                sap(out_t, 0, [[1, 12 * S]]),
                                sap(mt, 0, [[1, 12 * S]]), A.add)
                for si in (0, 1):
                    ob = (12 + si * 6) * S
                    eb = e[si] * 6 * S
                    dv = dj[si]
                    if dv.imag == 0.0:
                        op = A.add if dv.real > 0 else A.subtract
                        V.tensor_tensor(sap(out_t, ob, [[1, 6 * S]]),
                                        sap(out_t, ob, [[1, 6 * S]]),
                                        sap(mt, eb, [[1, 6 * S]]), op)
                    else:
                        sg = dv.imag > 0
                        # out_re += -sg*m_im ; out_im += sg*m_re
                        ore = [[2 * S, 3], [1, S]]
                        V.tensor_tensor(sap(out_t, ob, ore), sap(out_t, ob, ore),
                                        sap(mt, eb + S, ore),
                                        A.subtract if sg else A.add)
                        V.tensor_tensor(sap(out_t, ob + S, ore),
                                        sap(out_t, ob + S, ore),
                                        sap(mt, eb, ore),
                                        A.add if sg else A.subtract)

            def zshift(src_t, nplanes, dz, tag):
                """dst[t,z] = src[t, z+dz] (periodic), dz in {-1,+1}."""
                dt_ = pool.tile([R, nplanes * S], F16, tag=tag, bufs=2)
                C = V.tensor_copy
                if dz == +1:
                    C(sap(dt_, 0, [[S, nplanes], [Z, TS], [1, Z - 1]]),
                      sap(src_t, 1, [[S, nplanes], [Z, TS], [1, Z - 1]]))
                    C(sap(dt_, Z - 1, [[S, nplanes], [Z, TS], [1, 1]]),
                      sap(src_t, 0, [[S, nplanes], [Z, TS], [1, 1]]))
                else:
                    C(sap(dt_, 1, [[S, nplanes], [Z, TS], [1, Z - 1]]),
                      sap(src_t, 0, [[S, nplanes], [Z, TS], [1, Z - 1]]))
                    C(sap(dt_, 0, [[S, nplanes], [Z, TS], [1, 1]]),
                      sap(src_t, Z - 1, [[S, nplanes], [Z, TS], [1, 1]]))
                return dt_

            def yshift(src_t, nplanes, dy, tag):
                """dst[row (x,y)] = src[row (x, y+dy)] (periodic in y), via
                SBUF->SBUF DMA row shifts."""
                dt_ = pool.tile([R, nplanes * S], F16, tag=tag, bufs=2)
                D = nc.scalar.dma_start
                for g in range(nx):
                    b = g * Y
                    if dy == +1:
                        D(out=dt_[b:b + Y - 1], in_=src_t[b + 1:b + Y])
                        D(out=dt_[b + Y - 1:b + Y], in_=src_t[b:b + 1])
                    else:
                        D(out=dt_[b + 1:b + Y], in_=src_t[b:b + Y - 1])
                        D(out=dt_[b:b + 1], in_=src_t[b + Y - 1:b + Y])
                return dt_

            # ---------------- directions: software-pipelined stages -------
            # stage i emits: proj(i+2) | products+bsum(i) | comb+shift(i-1)
            # | expand(i-2), so DVE never stalls on shift DMAs or RMW chains.
            dirs_order = [(2, +1), (2, -1), (3, +1), (3, -1),
                          (1, +1), (1, -1), (0, +1), (0, -1)]
            st = {}

            def make_h(i):
                mu, sgn = dirs_order[i]
                spec = DIRSPEC[mu]
                fwd = sgn == +1
                cj = spec["c"] if fwd else tuple(-v for v in spec["c"])
                if mu == 2:
                    ht = proj(psi_al, Z, spec["B"], cj)
                    if not fwd:
                        ht = zshift(ht, 12, +1, "hsh")
                elif mu == 3:
                    ht = proj(psi_al, 0 if fwd else 2 * Z, spec["B"], cj)
                elif mu == 1:
                    ht = proj(psi_al, Z, spec["B"], cj)
                    if not fwd:
                        ht = yshift(ht, 12, +1, "hsh")
                else:
                    ht = proj(psi_xf if fwd else psi_xb, Z, spec["B"], cj)
                st[("h", i)] = ht

            def front(i):
                mu, sgn = dirs_order[i]
                fwd = sgn == +1
                if mu == 2:
                    g_t, gtoff = g_loc[2], Z
                elif mu == 3:
                    g_t, gtoff = g_loc[3], (0 if fwd else Z)
                elif mu == 1:
                    g_t, gtoff = g_loc[1], Z
                else:
                    g_t, gtoff = (g_xf if fwd else g_loc[0]), Z
                st[("p", i)] = su3_front(g_t, gtoff, fwd, st[("h", i)])

            def comb_shift(i):
                mu, sgn = dirs_order[i]
                fwd = sgn == +1
                mt = su3_comb(st[("p", i)], fwd)
                if fwd and mu == 2:
                    mt = zshift(mt, 12, -1, "msh")
                elif fwd and mu == 1:
                    mt = yshift(mt, 12, -1, "msh")
                st[("m", i)] = mt

            def expand_d(i):
                mu, sgn = dirs_order[i]
                spec = DIRSPEC[mu]
                dj = spec["d"] if sgn == +1 else tuple(-v for v in spec["d"])
                expand(st[("m", i)], spec["e"], dj)

            make_h(0)
            make_h(1)
            for i in range(8):
                if i + 2 < 8:
                    make_h(i + 2)
                front(i)
                if i >= 1:
                    comb_shift(i - 1)
                if i >= 2:
                    expand_d(i - 2)
            comb_shift(7)
            expand_d(6)
            expand_d(7)

            nc.scalar.dma_start(out=outp[r0:r0 + R], in_=out_t[:])
        ctx_pool.__exit__(None, None, None)
    return nc


# ---------------------------------------------------------------- host side
def _prep_core_inputs(fv, gv, t0):
    """fv: [X,Y,Z,T,3,4,2] f32 (c,s,ri). gv: [4,X,Y,Z,T,3,3,2] (r,c,ri).
    Returns fh [XY, 24*(TH*Z)] planes (s,c,ri) layout [t,z], and
    gg [4, XY, 18*(TG*Z)] planes (ri,r,c) of -0.5*U, both fp16."""
    Tl = T
    slots = [(t0 - 1) % Tl] + [(t0 + i) % Tl for i in range(TS)] + [(t0 + TS) % Tl]
    f = fv[:, :, :, slots]                       # [X,Y,Z,TH,c,s,ri]
    f = f.transpose(0, 1, 5, 4, 6, 3, 2)         # [X,Y,s,c,ri,TH,Z]
    fhn = np.ascontiguousarray(f, dtype=np.float16).reshape(XY, 24 * PP)
    gslots = [(t0 - 1 + i) % Tl for i in range(TG)]
    g = gv[:, :, :, :, gslots]                   # [4,X,Y,Z,TG,r,c,ri]
    g = g.transpose(0, 1, 2, 7, 5, 6, 4, 3)      # [4,X,Y,ri,r,c,TG,Z]
    ggn = np.ascontiguousarray(g, dtype=np.float32)
    ggn *= -0.5
    return fhn, ggn.astype(np.float16).reshape(4, XY, 18 * GP)


def _out_to_complex(o):
    o = o.astype(np.float32).reshape(X, Y, 4, 3, 2, TS, Z)   # [X,Y,s,c,ri,t,z]
    o = o.transpose(0, 1, 6, 5, 3, 2, 4)                     # [X,Y,Z,t,c,s,ri]
    return (o[..., 0] + 1j * o[..., 1]).astype(np.complex64)


def _build_in_maps(field, gauge_field):
    fv = np.ascontiguousarray(field).view(np.float32).reshape(X, Y, Z, T, 3, 4, 2)
    gv = np.ascontiguousarray(gauge_field).view(np.float32).reshape(4, X, Y, Z, T, 3, 3, 2)
    in_maps = []
    for k in range(NCORES):
        fhn, ggn = _prep_core_inputs(fv, gv, k * TS)
        in_maps.append({"fh": fhn, "gg": ggn})
    return in_maps


def kernel(field, gauge_field):
    from concourse.bass_utils import run_bass_kernel_spmd

    key = "full"
    if key not in _CACHE:
        _CACHE[key] = build_module()
    nc = _CACHE[key]

    in_maps = _build_in_maps(field, gauge_field)
    res = run_bass_kernel_spmd(nc, in_maps, list(range(NCORES))).results

    out = np.empty((X, Y, Z, T, 3, 4), np.complex64)
    for k in range(NCORES):
        out[:, :, :, k * TS:(k + 1) * TS] = _out_to_complex(res[k]["outp"])
    return out
